# revision 36
# baseline (speedup 1.0000x reference)
"""Trainium2 Bass kernel for a DeciLM SSM (Mamba2-style) decoder layer.

8-way tensor parallel over heads for the SSM; token-parallel out_proj.
  - Host folds ln_w+mup into in_proj_w, norm_w into out_proj_w, casts the
    big operands to bf16 and prepacks them k-tile-contiguous.
  - Device: RMSNorm applied post-matmul, single pass over hs (resident in
    SBUF as bf16), bf16 in_proj matmuls with full-K PSUM accumulation
    (x/B/C/dt first; the four z c-tiles are interleaved into the scan
    chunks to keep PE dense and hot), causal depthwise conv as shifted
    DVE MACs, Mamba2 chunked-SSD scan (Q=128) on PE with f32r decay
    matmuls, gated norm producing per-destination token slices, a single
    1MB AllToAll (instead of an 8.4MB AllGather), then a token-parallel
    out_proj: every core multiplies the FULL out_proj weight (32MB,
    prefetched/streamed) against its 128-token gated activations and the
    per-token inverse-rms scale is applied post-matmul.
  - Host reassembles the per-core token slices into the full output.
"""
import numpy as np
from ml_dtypes import bfloat16

import concourse.bass as bass
from concourse import bacc
import concourse.mybir as mybir
import concourse.tile as tile
from concourse import bass_utils
from concourse.masks import make_identity

F32 = mybir.dt.float32
F32R = mybir.dt.float32r
BF16 = mybir.dt.bfloat16
AF = mybir.ActivationFunctionType
ALU = mybir.AluOpType

H = 4096; DS = 4096; S = 128; NH = 64; HD = 64; KC = 4; EPS = 1e-5
L = 1024
NCORE = 8
HL = NH // NCORE            # 8 local heads
DSL = DS // NCORE           # 512 local channels
Q = 128; NQ = L // Q        # scan chunks
NKT = H // 128              # 32 contraction tiles
NMT = H // 128              # 32 out_proj row tiles
# A2A layout: block d = [128 rows(p), 4*Q cols(jj,t)]; channel = s*512+jj*128+p

# in_proj c-tiles computed in phase 1 (x | B | C | dt).
CT_X = [("x0", 128, 0), ("x1", 128, 1), ("B", 128, 4), ("C", 128, 5),
        ("x2", 128, 2), ("x3", 128, 3), ("dt", HL, 6)]
TOTWX = NKT * sum(m for _, m, _ in CT_X)
TOTWZ = NKT * 512           # four z c-tiles, done inside the scan
NWPRE = 14                  # out_proj weight tiles prefetched before A2A ends


def build_program(unroll=1, stop_after=None):
    nc = bacc.Bacc("TRN2", target_bir_lowering=False, debug=False,
                   num_devices=NCORE)
    hs_pk = nc.dram_tensor("hs_pk", [128, NKT * L], BF16, kind="ExternalInput")
    w_in = nc.dram_tensor("w_in", [128, TOTWX + TOTWZ], BF16,
                          kind="ExternalInput")
    w_out = nc.dram_tensor("w_out", [128, NMT * NKT * 128], BF16,
                           kind="ExternalInput")
    conv_w = nc.dram_tensor("conv_w", [DSL + 2 * S, KC], F32,
                            kind="ExternalInput")
    a_neg = nc.dram_tensor("a_neg", [HL, 1], F32, kind="ExternalInput")
    dt_bias = nc.dram_tensor("dt_bias", [HL, 1], F32, kind="ExternalInput")
    d_vec = nc.dram_tensor("d_vec", [128, DSL // 128], F32, kind="ExternalInput")
    outT = nc.dram_tensor("outT", [H, Q], F32, kind="ExternalOutput")

    with tile.TileContext(nc) as tc:
        for _ in range(unroll):
            _body(nc, tc, hs_pk, w_in, w_out, conv_w, a_neg, dt_bias, d_vec,
                  outT, stop_after=stop_after)
    nc.finalize()
    return nc


def _body(nc, tc, hs_pk, w_in, w_out, conv_w, a_neg, dt_bias, d_vec, outT,
          stop_after=None):
    from contextlib import ExitStack
    with ExitStack() as top:
        P = top.enter_context
        const = P(tc.tile_pool(name="const", bufs=1))
        dram = P(tc.tile_pool(name="dram", bufs=1, space="DRAM"))

        a2a_in = dram.tile([NCORE * 128, 4 * Q], BF16, name="a2ain")
        a2a_out = dram.tile([NCORE * 128, 4 * Q], BF16, name="a2aout")

        # ---- constants ----
        ident = const.tile([128, 128], F32)
        make_identity(nc, ident[:, :])
        ident_b = const.tile([128, 128], BF16)
        nc.vector.tensor_copy(ident_b[:, :], ident[:, :])
        ones_col_b = const.tile([128, 1], BF16)
        nc.vector.memset(ones_col_b[:, :], 1.0)
        ones_row = const.tile([1, 128], F32)
        nc.vector.memset(ones_row[:, :], 1.0)
        melt = const.tile([128, 128], F32)    # [s,t]: 0 if t>=s else -1e30
        nc.gpsimd.memset(melt[:, :], 0.0)
        nc.gpsimd.affine_select(out=melt[:, :], in_=melt[:, :],
                                compare_op=ALU.is_ge, fill=-1e30,
                                base=0, pattern=[[1, 128]],
                                channel_multiplier=-1)
        ones_row_b = const.tile([1, 128], BF16)
        nc.vector.memset(ones_row_b[:, :], 1.0)
        eps_sb = const.tile([128, 1], F32)
        nc.vector.memset(eps_sb[:, :], EPS)
        a_sb = const.tile([HL, 1], F32)
        nc.sync.dma_start(a_sb[:, :], a_neg[:, :])
        dtb_sb = const.tile([HL, 1], F32)
        nc.sync.dma_start(dtb_sb[:, :], dt_bias[:, :])
        dv_sb = const.tile([128, DSL // 128], F32)
        nc.sync.dma_start(dv_sb[:, :], d_vec[:, :])
        gmask = const.tile([HL, L], F32)
        nc.vector.memset(gmask[:, :], 1.0)
        nc.vector.memset(
            gmask[:, :].rearrange("p (q t) -> p q t", t=Q)[:, :, 0:1], 0.0)
        cw_sb = const.tile([128, 6 * KC], F32)
        for j in range(6):
            nc.sync.dma_start(cw_sb[:, j * KC:(j + 1) * KC],
                              conv_w[j * 128:(j + 1) * 128, :])

        NWOE = 2
        woe = const.tile([128, 0], F32) if False else None
        woep = P(tc.tile_pool(name="woep", bufs=1))
        woe = woep.tile([128, NWOE * NKT * 128], BF16)
        for m in range(NWOE):
            csl = slice(m * NKT * 128, (m + 1) * NKT * 128)
            nc.scalar.dma_start(woe[:, csl], w_out[:, csl])

        with ExitStack() as s14:
            live14 = s14.enter_context(tc.tile_pool(name="live14", bufs=1))
            hsp = s14.enter_context(tc.tile_pool(name="hs", bufs=1))
            wstr = s14.enter_context(tc.tile_pool(name="wstr", bufs=2))
            rsp = s14.enter_context(tc.tile_pool(name="rsp", bufs=1))

            hs_all = hsp.tile([128, NKT * L], BF16)
            rsb_in = rsp.tile([128, L], F32)
            sc = s14.enter_context(tc.tile_pool(name="scp", bufs=1))
            lca = sc.tile([HL, L], F32)
            lml = sc.tile([HL, L], F32)
            lmln = sc.tile([HL, L], F32)
            u_sb = sc.tile([HL, L], F32)
            dtsp = sc.tile([HL, L], F32)
            zx_z = live14.tile([128, 4 * L], BF16)     # z (gate input)
            y_sb = live14.tile([128, 4 * L], BF16)     # scan y
            xbc_stack = ExitStack()
            xbcp = xbc_stack.enter_context(tc.tile_pool(name="xbcp", bufs=1))
            zx_xbc = xbcp.tile([128, 6 * L], BF16)     # x0-3 | B | C
            dt_raw = live14.tile([HL, L], F32)
            convo = live14.tile([128, 6 * L], F32)     # silu(conv): x|B|C

            # ================= phase 1: stats + in_proj xBC/dt ===========
            with ExitStack() as s1:
                wk1 = s1.enter_context(tc.tile_pool(name="wk1", bufs=2))
                ps1 = s1.enter_context(
                    tc.tile_pool(name="ps1", bufs=3, space="PSUM"))
                ps1b = s1.enter_context(
                    tc.tile_pool(name="ps1b", bufs=1, space="PSUM"))

                ssq_ps0 = ps1b.tile([1, 512], F32, tag="ssq0")
                ssq_ps1 = ps1b.tile([1, 512], F32, tag="ssq1")
                ssq_ps = [ssq_ps0, ssq_ps1]
                for k4 in range(0, NKT, 4):
                    nc.sync.dma_start(hs_all[:, k4 * L:(k4 + 4) * L],
                                      hs_pk[:, k4 * L:(k4 + 4) * L])
                # token rms stats: sum of squares via ones-matmul
                for k in range(NKT):
                    hk = hs_all[:, k * L:(k + 1) * L]
                    sq = wk1.tile([128, L], BF16, tag="sq")
                    nc.vector.tensor_mul(sq[:, :], hk, hk)
                    for th in range(2):
                        nc.tensor.matmul(
                            ssq_ps[th][:, :], ones_col_b[:, :],
                            sq[:, th * 512:th * 512 + 512],
                            start=(k == 0), stop=(k == NKT - 1))
                for th in range(2):
                    tsl = slice(th * 512, (th + 1) * 512)
                    rs_row = wk1.tile([1, 512], F32, tag="rs")
                    nc.scalar.activation(rs_row[:, :], ssq_ps[th][:, :],
                                         AF.Sqrt, bias=eps_sb[0:1, 0:1],
                                         scale=1.0 / H)
                    nc.vector.reciprocal(rs_row[:, :], rs_row[:, :])
                    rsb_ps = ps1b.tile([128, 512], F32, tag="bc")
                    nc.tensor.matmul(rsb_ps[:, :], ones_row[:, :],
                                     rs_row[:, :])
                    nc.vector.tensor_copy(rsb_in[:, tsl], rsb_ps[:, :])

                base = 0
                for name, M, j in CT_X:
                    wt = wstr.tile([128, NKT * 128], BF16, tag="wt")
                    nc.sync.dma_start(wt[:, 0:NKT * M],
                                      w_in[:, base:base + NKT * M])
                    for th in range(2):
                        tsl = slice(th * 512, (th + 1) * 512)
                        zx_ps = ps1.tile([128, 512], F32, tag="mm")
                        for k in range(NKT):
                            nc.tensor.matmul(
                                zx_ps[0:M, :], wt[:, k * M:k * M + M],
                                hs_all[:, k * L + th * 512:
                                       k * L + th * 512 + 512],
                                start=(k == 0), stop=(k == NKT - 1))
                        if name == "dt":
                            nc.vector.tensor_mul(
                                dt_raw[:, tsl], zx_ps[0:M, :],
                                rsb_in[0:M, tsl])
                        else:
                            dsl_ = zx_xbc[0:M, j * L + th * 512:
                                          j * L + th * 512 + 512]
                            nc.vector.tensor_mul(dsl_, zx_ps[0:M, :],
                                                 rsb_in[0:M, tsl])
                    base += NKT * M

            if stop_after == "inproj":
                return
            # ===== phases 2-4: conv, dt, scan (+z interleave), gate, A2A ==
            with ExitStack() as s2:
                with ExitStack() as s2a:
                    wk2a = s2a.enter_context(tc.tile_pool(name="wk2a", bufs=1))
                    # ---- causal depthwise conv + silu ----
                    for j in [0, 1, 4, 5, 2, 3]:
                        src = zx_xbc[:, j * L:(j + 1) * L]
                        xpad = wk2a.tile([128, L + 4], F32, tag="xpad")
                        nc.vector.memset(xpad[:, 0:4], 0.0)
                        nc.vector.tensor_copy(xpad[:, 4:4 + L], src)
                        t0 = wk2a.tile([128, L], F32, tag="cv0")
                        t1 = wk2a.tile([128, L], F32, tag="cv1")
                        nc.vector.tensor_scalar_mul(
                            t0[:, :], xpad[:, 1:1 + L],
                            cw_sb[:, j * KC:j * KC + 1])
                        nc.vector.scalar_tensor_tensor(
                            t1[:, :], xpad[:, 2:2 + L],
                            cw_sb[:, j * KC + 1:j * KC + 2], t0[:, :],
                            op0=ALU.mult, op1=ALU.add)
                        nc.vector.scalar_tensor_tensor(
                            t0[:, :], xpad[:, 3:3 + L],
                            cw_sb[:, j * KC + 2:j * KC + 3], t1[:, :],
                            op0=ALU.mult, op1=ALU.add)
                        nc.vector.scalar_tensor_tensor(
                            t1[:, :], xpad[:, 4:4 + L],
                            cw_sb[:, j * KC + 3:j * KC + 4], t0[:, :],
                            op0=ALU.mult, op1=ALU.add)
                        nc.scalar.activation(convo[:, j * L:(j + 1) * L],
                                             t1[:, :], AF.Silu)

                    # ---- dt path ----
                    nc.scalar.activation(dtsp[:, :], dt_raw[:, :],
                                         AF.Exp, bias=dtb_sb[:, 0:1])
                    nc.scalar.activation(dtsp[:, :], dtsp[:, :], AF.Ln, bias=1.0)
                    logd = wk2a.tile([HL, L], F32, tag="cv0")
                    nc.vector.tensor_scalar_mul(logd[:, :], dtsp[:, :],
                                                a_sb[:, 0:1])
                    nc.vector.tensor_tensor_scan(lca[:, :], gmask[:, :],
                                                 logd[:, :], initial=0.0,
                                                 op0=ALU.mult, op1=ALU.add)
                    nc.scalar.activation(u_sb[:, :], lca[:, :], AF.Exp)
                    lnd = wk2a.tile([HL, L], F32, tag="cv1")
                    nc.scalar.activation(lnd[:, :], dtsp[:, :], AF.Ln)
                    nc.vector.tensor_sub(lml[:, :], lca[:, :], lnd[:, :])
                    nc.scalar.mul(lmln[:, :], lml[:, :], -1.0)

                # close conv inputs: frees zx_xbc for weight prefetch
                xbc_stack.close()

                # ---- chunked scan with z-projection interleave ----
                with ExitStack() as s2b:
                    scb = s2b.enter_context(tc.tile_pool(name="scb", bufs=1))
                    wvt_sb = scb.tile([128, NQ * HL], F32)
                    dqb_sb = scb.tile([128, NQ * HL], F32)
                    hst = scb.tile([128, 2 * HL * HD], BF16)
                    flat = scb.tile([1, 2 * L], F32)
                    ub_sb = scb.tile([128, L], BF16)
                    scb_fb = scb.tile([1, L], BF16)
                    scb_lt = scb.tile([128, HL], F32)
                    scb_arg = scb.tile([128, 1024], F32)
                    nc.vector.memset(hst[:, :], 0.0)
                    scw = s2b.enter_context(tc.tile_pool(name="scw", bufs=2))
                    # chunk decays dq[h,q] -> broadcast [128, HL*NQ]
                    dql = scb.tile([1, HL * NQ], F32)
                    nc.sync.dma_start(
                        dql[:, :],
                        u_sb[:, :].rearrange("p (q t) -> p q t", t=Q)
                        [:, :, Q - 1:Q])
                    dqb_ps0 = s2b.enter_context(
                        tc.tile_pool(name="psq", bufs=1, space="PSUM"))
                    dqb_ps = dqb_ps0.tile([128, HL * NQ], F32, tag="dq")
                    nc.tensor.matmul(dqb_ps[:, :], ones_row[:, :],
                                     dql[:, :])
                    nc.vector.tensor_copy(dqb_sb[:, :], dqb_ps[:, :])
                    wk2 = s2b.enter_context(tc.tile_pool(name="wk2", bufs=2))
                    ps_tp = s2b.enter_context(
                        tc.tile_pool(name="ps_tp", bufs=2, space="PSUM"))
                    ps_acc = s2b.enter_context(
                        tc.tile_pool(name="ps_acc", bufs=2, space="PSUM"))
                    ps_bb = s2b.enter_context(
                        tc.tile_pool(name="ps_bb", bufs=1, space="PSUM"))
                    ps_z = s2b.enter_context(
                        tc.tile_pool(name="ps_z", bufs=1, space="PSUM"))

                    for q in range(NQ):
                        qsl = slice(q * Q, (q + 1) * Q)
                        nc.sync.dma_start(flat[:, 0:HL * Q], lca[:, qsl])
                        nc.sync.dma_start(flat[:, HL * Q:2 * HL * Q],
                                          u_sb[:, qsl])
                        flat_b = scb_fb
                        nc.vector.tensor_copy(flat_b[:, :],
                                              flat[:, HL * Q:2 * HL * Q])
                        ub_ps = ps_bb.tile([128, 1024], F32, tag="bb")
                        for half in range(2):
                            nc.tensor.matmul(
                                ub_ps[:, half * 512:half * 512 + 512],
                                ones_row_b[:, :],
                                flat_b[:, half * 512:half * 512 + 512])
                        nc.vector.tensor_copy(ub_sb[:, :], ub_ps[:, :])

                        # batched decay matrix for all 8 heads:
                        # arg[s, h*128+t] = lca[h,t] - lml[h,s] + melt[s,t]
                        # row term lca[h,t] via one f32 broadcast matmul,
                        # then per-head (in0 - colscalar) + melt on DVE.
                        lml_tp = ps_tp.tile([128, 128], F32, tag="tp")
                        nc.tensor.transpose(lml_tp[0:128, 0:HL],
                                            lml[:, qsl], ident[0:HL, 0:HL])
                        lmlT = scb_lt
                        nc.scalar.copy(lmlT[:, :], lml_tp[0:128, 0:HL])
                        lca_ps = ps_bb.tile([128, 1024], F32, tag="bb")
                        for half in range(2):
                            nc.tensor.matmul(
                                lca_ps[:, half * 512:half * 512 + 512],
                                ones_row[:, :],
                                flat[:, half * 512:half * 512 + 512])
                        arg_all = scb_arg
                        for h in range(HL):
                            nc.vector.scalar_tensor_tensor(
                                arg_all[:, h * Q:(h + 1) * Q],
                                lca_ps[:, h * Q:(h + 1) * Q],
                                lmlT[:, h:h + 1], melt[:, :],
                                op0=ALU.subtract, op1=ALU.add)
                        exparg = scw.tile([128, 1024], BF16, tag="earg")
                        nc.scalar.activation(exparg[:, :], arg_all[:, :],
                                             AF.Exp)

                        w0_ps = ps_tp.tile([128, 128], F32, tag="tp")
                        nc.tensor.matmul(
                            w0_ps[:, :],
                            convo[:, 4 * L + q * Q:4 * L + (q + 1) * Q],
                            convo[:, 5 * L + q * Q:5 * L + (q + 1) * Q])
                        w0t = scw.tile([128, 128], F32, tag="w0t")
                        nc.scalar.copy(w0t[:, :], w0_ps[:, :])
                        bt_ps = ps_tp.tile([128, 128], F32, tag="tp")
                        nc.tensor.transpose(
                            bt_ps[:, :],
                            convo[:, 4 * L + q * Q:4 * L + (q + 1) * Q],
                            ident[:, :])
                        btok = scw.tile([128, Q], F32, tag="btok")
                        nc.scalar.copy(btok[:, :], bt_ps[:, :])
                        wv = wk2.tile([HL, Q], F32, tag="wv")
                        nc.vector.tensor_scalar(
                            out=wv[:, :], in0=lml[:, qsl],
                            scalar1=lca[:, q * Q + Q - 1:q * Q + Q],
                            scalar2=None, op0=ALU.subtract)
                        nc.scalar.activation(wv[:, :], wv[:, :], AF.Exp,
                                             scale=-1.0)
                        wv_ps = ps_tp.tile([128, 128], F32, tag="tp")
                        nc.tensor.transpose(wv_ps[0:128, 0:HL], wv[:, :],
                                            ident[0:HL, 0:HL])
                        nc.vector.tensor_copy(
                            wvt_sb[:, q * HL:(q + 1) * HL],
                            wv_ps[0:128, 0:HL])

                        # per-head-block wm/ctu muls (w0t/ctsb stay f32,
                        # single bf16 rounding at the product)
                        wm_all = scw.tile([128, 1024], BF16, tag="wma")
                        ctu_all = scw.tile([128, 1024], BF16, tag="ctua")
                        for h in range(HL):
                            hb = slice(h * Q, (h + 1) * Q)
                            nc.vector.tensor_mul(wm_all[:, hb],
                                                 exparg[:, hb], w0t[:, :])
                            nc.vector.tensor_mul(
                                ctu_all[:, hb], ub_sb[:, hb],
                                convo[:, 5 * L + q * Q:5 * L + (q + 1) * Q])

                        for h in range(HL):
                            hb = slice(h * Q, (h + 1) * Q)
                            xcol = slice((h // 2) * L + q * Q,
                                         (h // 2) * L + (q + 1) * Q)
                            if h % 2 == 0:
                                # transpose both heads of the pair at once
                                pcol = slice((h // 2) * L + q * Q,
                                             (h // 2) * L + (q + 1) * Q)
                                xt_ps = ps_tp.tile([128, 128], F32, tag="tp")
                                nc.tensor.transpose(xt_ps[:, :],
                                                    convo[0:128, pcol],
                                                    ident[:, :])
                                xpair = scw.tile([128, 128], BF16, tag="xpair")
                                nc.scalar.copy(xpair[:, :], xt_ps[:, :])
                            xtok = xpair[:, (h % 2) * 64:(h % 2) * 64 + 64]
                            wm = wm_all[:, hb]
                            ctu = ctu_all[:, hb]
                            hprev = hst[:, (2 * h + (q % 2)) * HD:
                                        (2 * h + (q % 2)) * HD + HD]
                            hnext = hst[:, (2 * h + ((q + 1) % 2)) * HD:
                                        (2 * h + ((q + 1) % 2)) * HD + HD]
                            if h % 2 == 0:
                                y_pair = ps_acc.tile([128, 128], F32,
                                                     tag="acc")
                            y_ps = y_pair[(h % 2) * 64:(h % 2) * 64 + 64, :]
                            nc.tensor.matmul(y_ps, xtok, wm,
                                             start=True, stop=False)
                            nc.tensor.matmul(y_ps, hprev, ctu,
                                             start=False, stop=True)
                            xw = scw.tile([128, 64], F32, tag="xw")
                            nc.gpsimd.tensor_scalar_mul(
                                xw[:, :], xtok,
                                wvt_sb[:, q * HL + h:q * HL + h + 1])
                            h_ps = ps_acc.tile([128, 64], F32, tag="acc")
                            nc.tensor.matmul(h_ps[:, :], btok[:, :],
                                             xw[:, :])
                            nc.vector.scalar_tensor_tensor(
                                hnext, hprev,
                                dqb_sb[:, h * NQ + q:h * NQ + q + 1],
                                h_ps[:, :], op0=ALU.mult, op1=ALU.add)
                            if h % 2 == 1:
                                # evacuate both heads of the pair at once
                                nc.vector.scalar_tensor_tensor(
                                    y_sb[0:128, xcol], convo[0:128, xcol],
                                    dv_sb[:, h // 2:h // 2 + 1],
                                    y_pair[:, :], op0=ALU.mult, op1=ALU.add)

                        # ---- z-projection units (all done by chunk 6,
                        # so the gate overlaps chunk 7) ----
                        zunits = {0: [0], 1: [1], 2: [2], 3: [3], 4: [4],
                                  5: [5], 6: [6], 7: [7]}[q]
                        for u in zunits:
                            zj, zth = u // 2, u % 2
                            if zth == 0:
                                wz = wstr.tile([128, NKT * 128], BF16,
                                               tag="wt", name="wz%d" % zj)
                                nc.sync.dma_start(
                                    wz[:, :],
                                    w_in[:, TOTWX + zj * NKT * 128:
                                         TOTWX + (zj + 1) * NKT * 128])
                            tsl = slice(zth * 512, (zth + 1) * 512)
                            z_ps = ps_z.tile([128, 512], F32, tag="zmm")
                            for k in range(NKT):
                                nc.tensor.matmul(
                                    z_ps[:, :], wz[:, k * 128:(k + 1) * 128],
                                    hs_all[:, k * L + zth * 512:
                                           k * L + zth * 512 + 512],
                                    start=(k == 0), stop=(k == NKT - 1))
                            nc.vector.tensor_mul(
                                zx_z[:, zj * L + zth * 512:
                                     zj * L + zth * 512 + 512],
                                z_ps[:, :], rsb_in[:, tsl])

                if stop_after == "scan":
                    return
                # ---- gate -> per-dest contiguous A2A blocks ----
                with ExitStack() as s2c:
                    wkg = s2c.enter_context(tc.tile_pool(name="wkg", bufs=2))
                    a2av = a2a_in[:, :].rearrange("(d p) c -> d p c", p=128)
                    for d in range(NCORE):
                        dsl_t = slice(d * Q, (d + 1) * Q)
                        gtd = wkg.tile([128, 4 * Q], BF16, tag="gtd")
                        for jj in range(4):
                            slz = wkg.tile([128, Q], BF16, tag="slz")
                            nc.scalar.activation(
                                slz[:, :],
                                zx_z[:, jj * L + d * Q:jj * L + (d + 1) * Q],
                                AF.Silu)
                            nc.vector.tensor_mul(
                                gtd[:, jj * Q:(jj + 1) * Q],
                                y_sb[:, jj * L + d * Q:jj * L + (d + 1) * Q],
                                slz[:, :])
                        nc.scalar.dma_start(a2av[d, :, :], gtd[:, :])
                    if stop_after != "noag":
                        nc.gpsimd.collective_compute(
                            "AllToAll", ALU.bypass,
                            replica_groups=[list(range(NCORE))],
                            ins=[a2a_in[:, :]],
                            outs=[a2a_out[:, :]],
                        )

        if stop_after == "scan":
            return
        # ================= phase 5: token-parallel out_proj ==============
        with ExitStack() as s5:
            gp = s5.enter_context(tc.tile_pool(name="gp", bufs=1))
            wk5 = s5.enter_context(tc.tile_pool(name="wk5", bufs=4))
            ot5 = s5.enter_context(tc.tile_pool(name="ot5", bufs=4))
            ps5 = s5.enter_context(tc.tile_pool(name="ps5", bufs=4, space="PSUM"))
            ps5b = s5.enter_context(
                tc.tile_pool(name="ps5b", bufs=2, space="PSUM"))

            # prefetch the first NWPRE out_proj weight tiles on the SP HWDGE
            # queue (no A2A dependency -> overlaps gate + collective)
            wo_pre = gp.tile([128, NWPRE * NKT * 128], BF16)
            for m in range(NWPRE):
                csl = slice((NWOE + m) * NKT * 128, (NWOE + m + 1) * NKT * 128)
                nc.scalar.dma_start(
                    wo_pre[:, m * NKT * 128:(m + 1) * NKT * 128], w_out[:, csl])

            gv = a2a_out[:, :].rearrange("(s p) c -> s p c", p=128)
            # gathered g: [128 part, (s jj) * Q + t] -- straight 2D loads
            g_sb = gp.tile([128, NKT * Q], BF16)
            for s in range(NCORE):
                nc.sync.dma_start(g_sb[:, s * 4 * Q:(s + 1) * 4 * Q],
                                  gv[s, :, :])
            # local per-token rms over all DS channels
            gsq = gp.tile([128, NKT * Q], BF16)
            nc.gpsimd.tensor_mul(gsq[:, :], g_sb[:, :], g_sb[:, :])
            rp = ps5b.tile([1, 128], F32, tag="rso")
            for k in range(NKT):
                nc.tensor.matmul(rp[:, :], ones_col_b[:, :],
                                 gsq[:, k * Q:(k + 1) * Q],
                                 start=(k == 0), stop=(k == NKT - 1))
            rso_row = gp.tile([1, Q], F32)
            nc.scalar.activation(rso_row[:, :], rp[:, :], AF.Sqrt,
                                 bias=eps_sb[0:1, 0:1], scale=1.0 / DS)
            nc.vector.reciprocal(rso_row[:, :], rso_row[:, :])
            rsb_out = gp.tile([128, Q], F32)
            bp = ps5b.tile([128, 128], F32, tag="bco")
            nc.tensor.matmul(bp[:, :], ones_row[:, :], rso_row[:, :])
            nc.vector.tensor_copy(rsb_out[:, :], bp[:, :])

            for m in range(NMT):
                if m < NWOE:
                    wsrc = woe[:, m * NKT * 128:(m + 1) * NKT * 128]
                elif m < NWOE + NWPRE:
                    wsrc = wo_pre[:, (m - NWOE) * NKT * 128:
                                  (m - NWOE + 1) * NKT * 128]
                else:
                    wtile = wk5.tile([128, NKT * 128], BF16, tag="wo")
                    nc.scalar.dma_start(
                        wtile[:, :],
                        w_out[:, m * NKT * 128:(m + 1) * NKT * 128])
                    wsrc = wtile[:, :]
                o_ps = ps5.tile([128, Q], F32, tag="mm")
                for k in range(NKT):
                    nc.tensor.matmul(
                        o_ps[:, :], wsrc[:, k * 128:(k + 1) * 128],
                        g_sb[:, k * Q:(k + 1) * Q],
                        start=(k == 0), stop=(k == NKT - 1))
                ot = ot5.tile([128, Q], F32, tag="ot")
                nc.vector.tensor_mul(ot[:, :], o_ps[:, :], rsb_out[:, :])
                nc.sync.dma_start(outT[m * 128:(m + 1) * 128, :], ot[:, :])


_NC_CACHE = {}


def get_program(unroll=1):
    if unroll not in _NC_CACHE:
        _NC_CACHE[unroll] = build_program(unroll)
    return _NC_CACHE[unroll]


def _pack_ktiles(a):
    """[4096, M] f32 -> [128, NKT*M] bf16, k-tile-contiguous."""
    m = a.shape[1]
    return np.ascontiguousarray(
        a.reshape(NKT, 128, m).transpose(1, 0, 2).reshape(128, NKT * m)
    ).astype(bfloat16)


def make_in_maps(inputs):
    hs = np.ascontiguousarray(np.asarray(inputs["hidden_states"],
                                         np.float32)[0])
    ln_w = np.asarray(inputs["ln_w"], np.float32)
    mup = np.asarray(inputs["mup_vector"], np.float32)
    w_in_full = (np.asarray(inputs["in_proj_w"], np.float32)
                 * ln_w[:, None] * mup[None, :])
    w_out_full = (np.asarray(inputs["out_proj_w"], np.float32)
                  * np.asarray(inputs["norm_w"], np.float32)[:, None])
    A = -np.exp(np.asarray(inputs["A_log"], np.float32))
    dtb = np.asarray(inputs["dt_bias"], np.float32)
    Dv = np.asarray(inputs["D"], np.float32)
    cw = np.asarray(inputs["conv_w"], np.float32)

    hs_pk = _pack_ktiles(np.ascontiguousarray(hs.T))
    # full out_proj weight, identical on every core:
    # [128 k-part, (m*NKT + k)*128 + col]
    w_out_pk = np.ascontiguousarray(
        w_out_full.reshape(NKT, 128, NMT, 128).transpose(1, 2, 0, 3)
        .reshape(128, NMT * NKT * 128)).astype(bfloat16)

    in_maps = []
    for c in range(NCORE):
        # absolute w_in column ranges per c-tile: CT_X order then z0..z3
        cols = []
        for name, M, j in CT_X:
            if name.startswith("x"):
                base = DS + c * DSL + j * 128
            elif name == "B":
                base = 2 * DS
            elif name == "C":
                base = 2 * DS + S
            else:  # dt
                base = 2 * DS + 2 * S + c * HL
            cols.append(np.arange(base, base + M))
        for j in range(4):
            base = c * DSL + j * 128
            cols.append(np.arange(base, base + 128))
        w_in_pk = np.concatenate(
            [_pack_ktiles(w_in_full[:, cs]) for cs in cols], axis=1)
        conv_rows = np.r_[np.arange(c * DSL, (c + 1) * DSL),
                          DS + np.arange(2 * S)]
        dmat = np.empty((128, DSL // 128), np.float32)
        for j in range(DSL // 128):
            dmat[0:64, j] = Dv[c * HL + 2 * j]
            dmat[64:128, j] = Dv[c * HL + 2 * j + 1]
        in_maps.append({
            "hs_pk": hs_pk,
            "w_in": w_in_pk,
            "w_out": w_out_pk,
            "conv_w": np.ascontiguousarray(cw[conv_rows]),
            "a_neg": np.ascontiguousarray(A[c * HL:(c + 1) * HL, None]),
            "dt_bias": np.ascontiguousarray(dtb[c * HL:(c + 1) * HL, None]),
            "d_vec": dmat,
        })
    return in_maps


def assemble(results, inputs):
    out = np.concatenate([r["outT"] for r in results], axis=1).T[None]
    residual = np.asarray(inputs["residual"], np.float32)
    return np.ascontiguousarray(out).astype(np.float32), residual


def kernel(**inputs):
    nc = get_program()
    in_maps = make_in_maps(inputs)
    res = bass_utils.run_bass_kernel_spmd(nc, in_maps,
                                          core_ids=list(range(NCORE)))
    return assemble(res.results, inputs)
